# revision 2
# baseline (speedup 1.0000x reference)
"""Trainium2 Bass kernel for GNN edge-softmax attention message passing.

v2 design (single-gather + on-chip q-expansion):
  - edges sorted by (core, superblock of 8 64-row blocks, col-chunk, block);
    8 cores own contiguous 12544-row slices
  - ONE plain dma_gather per call fetches per-edge col data (768B rows):
    [k fp16 | eigs fp16 | vhi bf16 | one bf16] — calls span all blocks of a
    superblock within one col-chunk, so ~240 calls/core instead of ~1600
  - per-edge q never gathered: per 128-edge tile, the one-hot localrow
    matrix (DVE is_eq) is transposed on TensorE and multiplied with the
    SBUF-resident per-block Q^T/eigs^T table, expanding q to per-edge rows
  - edge scores = DVE elementwise mult + reduce of (q-expanded, gathered k)
    in fp16; exp on ACT; A = [oh*e0 | oh*e1] on DVE
  - per tile one bf16 matmul accumulates [P0|d0 ; P1|d1] into one of 8
    per-block PSUM accumulators; whole superblock flushed with one copy+DMA
  - host: 0.5*(P0/d0 + P1/d1) combine
"""

import os
import sys
import types

import numpy as np

N = 100000
E = 3200000
H = 128
ED = 16
P6 = 6
NCORES = 8
R = 64                   # rows per block
CORE_ROWS = 12544
NPAD = CORE_ROWS * NCORES
NBLK = CORE_ROWS // R    # 196
SBB = 4                  # blocks per superblock (196 = 4*49); each block's
                         # PSUM accumulator owns a full 2KB bank (start_
                         # tensor_calc zeroes whole banks)
NCHUNK = 4
CHUNK = NPAD // NCHUNK   # 25088
ROW_B = 768              # gathered row bytes
FEAT = 144               # k(128) + eigs(16) fp16 features
MAXCALL = 1024           # 8 tiles/call
SUBT = 6                 # tiles per PSUM sub-batch
LAST_EXEC_NS = None


def _install_axon_hooks():
    if "antenv.axon_hooks" in sys.modules:
        return
    mod = types.ModuleType("antenv.axon_hooks")
    _hook = [None]
    mod.set_axon_ntff_profile_hook = lambda h: _hook.__setitem__(0, h)
    mod.get_axon_ntff_profile_hook = lambda: _hook[0]
    sys.modules["antenv.axon_hooks"] = mod
    try:
        import antenv
        antenv.axon_hooks = mod
    except ImportError:
        pass
    try:
        from trn_agent_boot.trn_boot import _ntff_profile_via_ctypes
        h = _ntff_profile_via_ctypes("/opt/axon/libaxon_pjrt.so")
        if h is not None:
            mod.set_axon_ntff_profile_hook(h)
    except Exception:
        pass


NSB_CEIL = (NBLK + SBB - 1) // SBB  # 25 superblocks (last has 4 blocks)


def _prep(indices, path_type, wvals):
    """Sort/pad edges; build the shared call plan and per-core metadata.

    Returns (calls, T, CW, segs, per_core) where calls[i] =
    (chunk, n, tiles, flush) with tiles = [(blk, global_first, global_last)]
    per 128-edge tile, flush = list of sb to flush after this call.
    """
    row = indices[0].astype(np.int64)
    col = indices[1].astype(np.int64)
    core = row // CORE_ROWS
    lrow = row % CORE_ROWS
    blk = lrow // R
    sb = blk // SBB
    c = col // CHUNK
    # sort key: (core, sb, chunk, blk)
    key = ((core * NSB_CEIL + sb) * NCHUNK + c) * NBLK + blk
    order = np.argsort(key, kind="stable")
    row_s, col_s, pt_s = row[order], col[order], path_type[order]
    key_s = key[order]

    # group counts per (core, blk, chunk)
    gkey = (core * NBLK + blk) * NCHUNK + c
    counts = np.bincount(gkey[order], minlength=NCORES * NBLK * NCHUNK) \
        .reshape(NCORES, NBLK, NCHUNK)
    gmax = counts.max(axis=0)                      # [NBLK, NCHUNK]
    gpad = ((gmax + 127) // 128) * 128

    # first/last global tile index of each block, call plan
    calls = []
    tiles_flat = []  # (blk) per tile
    blk_first = {}
    blk_last = {}
    tpos = 0
    for s in range(NSB_CEIL):
        blks = range(s * SBB, min((s + 1) * SBB, NBLK))
        for cc in range(NCHUNK):
            # segments (blk, ntiles) in this (sb, chunk)
            run = [(b, int(gpad[b, cc]) // 128) for b in blks
                   if gpad[b, cc] > 0]
            # split into calls of <= MAXCALL idx (MAXCALL//128 tiles)
            pend = []
            pend_nt = 0
            for b, nt in run:
                while nt > 0:
                    take = min(nt, MAXCALL // 128 - pend_nt)
                    pend.append((b, take))
                    pend_nt += take
                    nt -= take
                    if pend_nt == MAXCALL // 128:
                        calls.append([cc, pend_nt * 128, pend, []])
                        pend = []
                        pend_nt = 0
            if pend:
                calls.append([cc, pend_nt * 128, pend, []])
        # record tile spans
        # (done below via tiles_flat once calls exist)
    # annotate per-tile block ids and block first/last tiles
    for ci, (cc, n, segs_c, fl) in enumerate(calls):
        for b, nt in segs_c:
            for _ in range(nt):
                if b not in blk_first:
                    blk_first[b] = tpos
                blk_last[b] = tpos
                tiles_flat.append(b)
                tpos += 1
    T = tpos
    CW = T * 8  # n/16 idx cols per tile = 128/16 = 8
    # flush: after the call containing the last tile of the last block of
    # each sb (all blocks of the sb are done by then since order is
    # (sb, chunk, blk))
    tile_of_call = []
    tacc = 0
    for ci, (cc, n, segs_c, fl) in enumerate(calls):
        tile_of_call.append(tacc)
        tacc += n // 128
    for s in range(NSB_CEIL):
        blks = [b for b in range(s * SBB, min((s + 1) * SBB, NBLK))
                if b in blk_last]
        if not blks:
            continue
        last_t = max(blk_last[b] for b in blks)
        # find call containing last_t
        ci = max(i for i, t0 in enumerate(tile_of_call) if t0 <= last_t)
        calls[ci][3].append(s)

    # meta segmentation for SBUF (break at call boundaries, ~384 tiles)
    segs = []
    lo = 0
    t_lo = 0
    acc = 0
    for i, (cc, n, segs_c, fl) in enumerate(calls):
        acc += n // 128
        if acc >= 384 or i + 1 == len(calls):
            segs.append((lo, i + 1, t_lo, acc))
            lo = i + 1
            t_lo += acc
            acc = 0

    # per-core metadata (groups appear in sorted (core, sb, chunk, blk)
    # order, so track a running position instead of a (blk, chunk) cumsum)
    base = np.concatenate(([0], np.cumsum(np.bincount(core, minlength=NCORES))))

    from ml_dtypes import bfloat16
    per_core = []
    for cr in range(NCORES):
        cidx = np.zeros((128, CW), np.int16)
        roff = np.full((128, T), -1.0, np.float16)
        e1b = np.zeros((128, T), bfloat16)
        # walk groups in (sb, chunk, blk) order and fill tile-columns
        # sequentially (matches call construction and sorted edge order)
        tcur = 0
        wcur = 0
        pos = int(base[cr])
        for s in range(NSB_CEIL):
            blks = range(s * SBB, min((s + 1) * SBB, NBLK))
            for cc in range(NCHUNK):
                for b in blks:
                    gsz = int(gpad[b, cc])
                    n_real = int(counts[cr, b, cc])
                    st = pos
                    pos += n_real
                    if gsz == 0:
                        continue
                    ccol = np.zeros(gsz, np.int64)
                    cro = np.full(gsz, -1.0, np.float32)
                    ce1 = np.zeros(gsz, np.float32)
                    ccol[:n_real] = col_s[st:st + n_real] % CHUNK
                    cro[:n_real] = (row_s[st:st + n_real] % CORE_ROWS) % R
                    ce1[:n_real] = wvals[pt_s[st:st + n_real]]
                    nt = gsz // 128
                    cidx[:, wcur:wcur + gsz // 16] = np.tile(
                        ccol.reshape(gsz // 16, 16).T.astype(np.int16),
                        (8, 1))
                    roff[:, tcur:tcur + nt] = \
                        cro.reshape(nt, 128).T.astype(np.float16)
                    e1b[:, tcur:tcur + nt] = \
                        ce1.reshape(nt, 128).T.astype(bfloat16)
                    tcur += nt
                    wcur += gsz // 16
        per_core.append(dict(cidx=cidx, roff=roff, e1b=e1b))
    return calls, T, CW, segs, per_core


def _build(calls, T, CW, segs):
    import concourse.mybir as mybir
    import concourse.tile as tile
    from concourse import bacc

    SEG_T = max(s[3] for s in segs)
    SEG_W = SEG_T * 8
    NT = MAXCALL // 128

    nc = bacc.Bacc(trn_type="TRN2", num_swdge_queues=4)
    kv = nc.dram_tensor("kv", [NPAD, ROW_B], mybir.dt.uint8,
                        kind="ExternalInput")
    qe = nc.dram_tensor("qe", [R, NBLK * FEAT], mybir.dt.float16,
                        kind="ExternalInput")
    ident = nc.dram_tensor("ident", [128, 128], mybir.dt.float16,
                           kind="ExternalInput")
    cidx = nc.dram_tensor("cidx", [128, CW], mybir.dt.int16,
                          kind="ExternalInput")
    roff = nc.dram_tensor("roff", [128, T], mybir.dt.float16,
                          kind="ExternalInput")
    e1b = nc.dram_tensor("e1b", [128, T], mybir.dt.uint16,
                         kind="ExternalInput")
    iota = nc.dram_tensor("iota", [128, R], mybir.dt.float16,
                          kind="ExternalInput")
    raw = nc.dram_tensor("raw", [NBLK * 128, 129], mybir.dt.float32,
                         kind="ExternalOutput")

    with tile.TileContext(nc) as tc:
        with tc.tile_pool(name="const", bufs=1) as cpool, \
             tc.tile_pool(name="meta", bufs=3) as meta, \
             tc.tile_pool(name="gpool", bufs=6) as gpool, \
             tc.tile_pool(name="work", bufs=4) as work, \
             tc.tile_pool(name="ohtp", bufs=1, space="PSUM") as ohtp, \
             tc.tile_pool(name="qxp", bufs=1, space="PSUM") as qxp, \
             tc.tile_pool(name="bpp", bufs=1, space="PSUM") as bpp:
            iota_t = cpool.tile([128, R], mybir.dt.float16)
            nc.sync.dma_start(out=iota_t[:], in_=iota[:, :])
            qe_t = cpool.tile([R, NBLK * FEAT], mybir.dt.float16)
            nc.sync.dma_start(out=qe_t[:], in_=qe[:, :])
            id_t = cpool.tile([128, 128], mybir.dt.float16)
            nc.sync.dma_start(out=id_t[:], in_=ident[:, :])

            seg_iter = iter(segs)
            cur_seg = None
            tpos = 0       # global tile index
            seg_t0 = 0
            wpos = 0
            bps = None
            bps_sb = None
            started = set()
            for ci, (cc, n, segs_c, flushes) in enumerate(calls):
                if cur_seg is None or ci >= cur_seg[1]:
                    cur_seg = next(seg_iter)
                    (clo, chi, t_lo, nt_seg) = cur_seg
                    seg_t0 = t_lo
                    nw_seg = nt_seg * 8
                    cidx_t = meta.tile([128, SEG_W], mybir.dt.int16,
                                       tag="cidx")
                    nc.sync.dma_start(out=cidx_t[:, :nw_seg],
                                      in_=cidx[:, t_lo * 8:t_lo * 8 + nw_seg])
                    roff_t = meta.tile([128, SEG_T], mybir.dt.float16,
                                       tag="roff")
                    nc.sync.dma_start(out=roff_t[:, :nt_seg],
                                      in_=roff[:, t_lo:t_lo + nt_seg])
                    e1b_t = meta.tile([128, SEG_T], mybir.dt.uint16,
                                      tag="e1b")
                    nc.sync.dma_start(out=e1b_t[:, :nt_seg],
                                      in_=e1b[:, t_lo:t_lo + nt_seg])
                    e1v = e1b_t[:].bitcast(mybir.dt.bfloat16)

                nt = n // 128
                st = tpos - seg_t0          # tile offset within seg
                sw = st * 8

                kg = gpool.tile([128, NT * ROW_B], mybir.dt.uint8, tag="kg")
                nc.gpsimd.dma_gather(
                    out_ap=kg[:, :nt * ROW_B].rearrange(
                        "p (n d) -> p n d", d=ROW_B),
                    in_ap=kv[cc * CHUNK:(cc + 1) * CHUNK, :],
                    idxs_ap=cidx_t[:, sw:sw + n // 16],
                    num_idxs=n, num_idxs_reg=n, elem_size=ROW_B,
                    queue_num=ci % 4)
                kf16 = kg[:].bitcast(mybir.dt.float16)   # [128, NT*384]
                kb16 = kg[:].bitcast(mybir.dt.bfloat16)

                # one-hot localrow for the whole call (fp16)
                oh = work.tile([128, NT * R], mybir.dt.float16, tag="oh")
                ohv = oh[:].rearrange("p (n d) -> p n d", d=R)
                nc.vector.tensor_tensor(
                    out=ohv[:, :nt, :],
                    in0=iota_t[:].rearrange("p (o d) -> p o d", o=1)
                        .to_broadcast([128, nt, R]),
                    in1=roff_t[:, st:st + nt].rearrange(
                        "p (n o) -> p n o", o=1).to_broadcast([128, nt, R]),
                    op=mybir.AluOpType.is_equal)

                if bps is None:
                    # one full 2KB bank per block accumulator
                    bps = bpp.tile([128, SBB * 512], mybir.dt.float32,
                                   tag="bps")
                    started = set()

                # per-tile block ids for this call
                tb = []
                for b, bnt in segs_c:
                    tb += [b] * bnt

                A = work.tile([128, NT * 2 * R], mybir.dt.bfloat16, tag="A")
                Av = A[:].rearrange("p (n d) -> p n d", d=2 * R)

                # q-expansion in PSUM sub-batches; scores batched call-wide
                qx16 = work.tile([128, (MAXCALL // 128) * FEAT],
                                 mybir.dt.float16, tag="qx16")
                for s0 in range(0, nt, SUBT):
                    s1 = min(s0 + SUBT, nt)
                    ns = s1 - s0
                    # transpose one-hot tiles -> [R, e] fp16 PSUM
                    # (256B regions, bank-contained)
                    ohT_ps = ohtp.tile([R, SUBT * 128], mybir.dt.float16,
                                       tag="ohT")
                    for j in range(s0, s1):
                        nc.tensor.transpose(
                            out=ohT_ps[:, (j - s0) * 128:(j - s0 + 1) * 128],
                            in_=ohv[:, j, :],
                            identity=id_t[:])
                    ohT_sb = work.tile([R, SUBT * 128], mybir.dt.float16,
                                       tag="ohTs")
                    nc.vector.tensor_copy(out=ohT_sb[:, :ns * 128],
                                          in_=ohT_ps[:, :ns * 128])
                    # q-expansion matmuls -> 1024B-strided regions so no
                    # matmul output crosses a PSUM bank boundary
                    qx_ps = qxp.tile([128, SUBT * 256], mybir.dt.float32,
                                     tag="qx")
                    for j in range(s0, s1):
                        b = tb[j]
                        nc.tensor.matmul(
                            out=qx_ps[:, (j - s0) * 256:(j - s0) * 256 + FEAT],
                            lhsT=ohT_sb[:, (j - s0) * 128:(j - s0 + 1) * 128],
                            rhs=qe_t[:, b * FEAT:(b + 1) * FEAT],
                            start=True, stop=True)
                    nc.scalar.copy(
                        out=qx16[:, s0 * FEAT:s1 * FEAT].rearrange(
                            "p (n d) -> p n d", d=FEAT),
                        in_=qx_ps[:].rearrange(
                            "p (n d) -> p n d", d=256)[:, :ns, 0:FEAT])
                # edge scores: prod + reduce (fp16 in, f32 accum out)
                prod = work.tile([128, (MAXCALL // 128) * FEAT],
                                 mybir.dt.float16, tag="prod")
                nc.vector.tensor_tensor(
                    out=prod[:, :nt * FEAT].rearrange(
                        "p (n d) -> p n d", d=FEAT),
                    in0=qx16[:, :nt * FEAT].rearrange(
                        "p (n d) -> p n d", d=FEAT),
                    in1=kf16.rearrange("p (n d) -> p n d", d=384)
                        [:, :nt, 0:FEAT],
                    op=mybir.AluOpType.mult)
                s0t = work.tile([128, MAXCALL // 128], mybir.dt.float32,
                                tag="s0t")
                nc.vector.tensor_reduce(
                    out=s0t[:, :nt],
                    in_=prod[:, :nt * FEAT].rearrange(
                        "p (n d) -> p n d", d=FEAT),
                    axis=mybir.AxisListType.X, op=mybir.AluOpType.add)
                e0 = work.tile([128, MAXCALL // 128], mybir.dt.bfloat16,
                               tag="e0")
                nc.scalar.activation(
                    out=e0[:, :nt], in_=s0t[:, :nt],
                    func=mybir.ActivationFunctionType.Exp)
                # A = [oh*e0 | oh*e1]
                nc.vector.tensor_tensor(
                    out=Av[:, :nt, 0:R],
                    in0=ohv[:, :nt, :],
                    in1=e0[:, :nt].rearrange("p (n o) -> p n o", o=1)
                        .to_broadcast([128, nt, R]),
                    op=mybir.AluOpType.mult)
                nc.vector.tensor_tensor(
                    out=Av[:, :nt, R:2 * R],
                    in0=ohv[:, :nt, :],
                    in1=e1v[:, st:st + nt].rearrange(
                        "p (n o) -> p n o", o=1).to_broadcast([128, nt, R]),
                    op=mybir.AluOpType.mult)
                # scatter matmuls: each block accumulates in its own bank
                for j in range(nt):
                    b = tb[j]
                    bo = b % SBB
                    first = b not in started
                    started.add(b)
                    nc.tensor.matmul(
                        out=bps[:, bo * 512:bo * 512 + 129],
                        lhsT=Av[:, j, :],
                        rhs=kb16.rearrange("p (n d) -> p n d", d=384)
                            [:, j, FEAT:FEAT + 129],
                        start=first,
                        stop=(tpos + j == _blk_last_tile(calls, b)))
                tpos += nt

                for sflush in flushes:
                    b0 = sflush * SBB
                    nb = min(SBB, NBLK - b0)
                    ev = work.tile([128, SBB * 129], mybir.dt.float32,
                                   tag="ev")
                    nc.scalar.copy(
                        out=ev[:, :nb * 129].rearrange(
                            "p (j c) -> p j c", c=129),
                        in_=bps[:].rearrange(
                            "p (j c) -> p j c", c=512)[:, :nb, 0:129])
                    nc.sync.dma_start(
                        out=raw[b0 * 128:(b0 + nb) * 128, :].rearrange(
                            "(j p) c -> p j c", p=128),
                        in_=ev[:, :nb * 129].rearrange(
                            "p (j c) -> p j c", c=129))
                    bps = None
    nc.finalize()
    return nc


_BLK_LAST = {}


def _blk_last_tile(calls, b):
    if not _BLK_LAST:
        t = 0
        for cc, n, segs_c, fl in calls:
            for bb, nt in segs_c:
                for _ in range(nt):
                    _BLK_LAST[bb] = t
                    t += 1
    return _BLK_LAST[b]


def kernel(q, k, v, eigs, lambda0, path_emb_w, indices, path_type):
    _install_axon_hooks()
    global _BLK_LAST
    _BLK_LAST = {}
    q = np.asarray(q, np.float32)
    k = np.asarray(k, np.float32)
    v = np.asarray(v, np.float32)
    eigs = np.asarray(eigs, np.float32)
    lambda0 = np.asarray(lambda0, np.float32)
    path_emb_w = np.asarray(path_emb_w, np.float32)
    indices = np.asarray(indices, np.int32)
    path_type = np.asarray(path_type, np.int32)

    from ml_dtypes import bfloat16
    ew = float(np.exp(lambda0[0]))
    wvals = np.exp(path_emb_w[:, 0]).astype(np.float32)

    calls, T, CW, segs, per_core = _prep(indices, path_type, wvals)

    # kv table row (768B): [k fp16 | eigs fp16 | vhi bf16 | one bf16 | pad]
    kv = np.zeros((NPAD, ROW_B), np.uint8)
    kv[:N, 0:256] = k.astype(np.float16).view(np.uint8)
    kv[:N, 256:288] = eigs.astype(np.float16).view(np.uint8)
    kv[:N, 288:544] = v.astype(bfloat16).view(np.uint8)
    kv[:, 544:546] = np.tile(
        np.array([1.0], bfloat16).view(np.uint8), (NPAD, 1))

    # per-core q/eigs tables [R, NBLK*FEAT]: block b cols hold q^T for rows
    # b*64..b*64+63 (prescaled)
    qs = (q * (1.0 / np.sqrt(np.float32(H)))).astype(np.float16)
    es = (eigs * ew).astype(np.float16)
    qe_full = np.zeros((NPAD, FEAT), np.float16)
    qe_full[:N, 0:H] = qs
    qe_full[:N, H:FEAT] = es

    ident = np.eye(128, dtype=np.float16)
    iota = np.tile(np.arange(R, dtype=np.float16), (128, 1))

    nc = _build(calls, T, CW, segs)

    in_maps = []
    for cr in range(NCORES):
        pc = per_core[cr]
        qe_core = qe_full[cr * CORE_ROWS:(cr + 1) * CORE_ROWS]  # [12544,144]
        qe_t = qe_core.reshape(NBLK, R, FEAT).transpose(1, 0, 2).reshape(
            R, NBLK * FEAT)
        in_maps.append({
            "kv": kv,
            "qe": np.ascontiguousarray(qe_t),
            "ident": ident,
            "cidx": pc["cidx"],
            "roff": pc["roff"],
            "e1b": pc["e1b"].view(np.uint16),
            "iota": iota,
        })

    from concourse.bass_utils import run_bass_kernel_spmd
    want_trace = bool(os.environ.get("KERNEL_TRACE"))
    res = run_bass_kernel_spmd(nc, in_maps, core_ids=list(range(NCORES)),
                               trace=want_trace)
    global LAST_EXEC_NS
    LAST_EXEC_NS = res.exec_time_ns
    if os.environ.get("KERNEL_DEBUG_RAW"):
        np.savez("/tmp/dbg_raw.npz", raw0=res.results[0]["raw"],
                 cidx0=per_core[0]["cidx"], roff0=per_core[0]["roff"],
                 e1b0=np.asarray(per_core[0]["e1b"], np.float32))

    out = np.zeros((NPAD, H), np.float32)
    for cr in range(NCORES):
        rawb = res.results[cr]["raw"].reshape(NBLK, 128, 129)
        p0 = rawb[:, 0:64, 0:128]
        p1 = rawb[:, 64:128, 0:128]
        d0 = rawb[:, 0:64, 128]
        d1 = rawb[:, 64:128, 128]
        d0 = np.where(d0 > 0, d0, 1.0)
        d1 = np.where(d1 > 0, d1, 1.0)
        blkout = 0.5 * (p0 / d0[..., None] + p1 / d1[..., None])
        out[cr * CORE_ROWS:(cr + 1) * CORE_ROWS] = blkout.reshape(CORE_ROWS, H)
    return out[:N]


if __name__ == "__main__":
    rng = np.random.default_rng(0)
    Et = int(os.environ.get("ET", "400000"))
    idx = rng.integers(0, N, size=(2, Et)).astype(np.int32)
    pt = rng.integers(0, P6, size=(Et,)).astype(np.int32)
    qq = rng.standard_normal((N, H), dtype=np.float32)
    kk = rng.standard_normal((N, H), dtype=np.float32)
    vv = rng.standard_normal((N, H), dtype=np.float32)
    ee = rng.standard_normal((N, ED), dtype=np.float32)
    l0 = np.zeros(1, np.float32)
    pw = rng.standard_normal((P6, 1), dtype=np.float32)

    out = kernel(qq, kk, vv, ee, l0, pw, idx, pt)

    row, col = idx[0], idx[1]
    x = (qq[row] * kk[col]).sum(-1) / np.sqrt(H) + np.exp(l0[0]) * (
        ee[row] * ee[col]).sum(-1)
    s1 = pw[pt, 0]
    exp0 = np.exp(x)
    d0 = np.zeros(N); np.add.at(d0, row, exp0)
    exp1 = np.exp(s1)
    d1 = np.zeros(N); np.add.at(d1, row, exp1)
    a = 0.5 * (exp0 / np.where(d0 == 0, 1, d0)[row] +
               exp1 / np.where(d1 == 0, 1, d1)[row])
    ref = np.zeros((N, H), np.float32)
    np.add.at(ref, row, a[:, None] * vv[col])
    num = np.linalg.norm(out - ref)
    den = np.linalg.norm(ref)
    print("rel err:", num / den)
